# revision 1
# baseline (speedup 1.0000x reference)
"""LDPC encoder kernel for Trainium2 (8 NeuronCores, batch-sharded).

Computes out = 1 - 2*((m @ G^T) mod 2)  (BPSK-mapped LDPC codeword).

  m: [16384, 1200] int32 (0/1)   G: [2400, 1200] float32 (0/1)
  out: [16384, 2400] float32 (+-1)

Strategy:
  - Shard the batch over 8 cores (2048 rows each); G replicated.
  - G is systematic (G[:1200] == I), so out[:, :1200] = 1 - 2*m is a pure
    elementwise map; only the 1200 parity columns need a matmul.
  - Matmul in bf16 (values 0/1/2 are exact; PSUM accumulates fp32 exactly).
    Host feeds m transposed ([K,B] layout) so the stationary operand needs
    no on-device transpose, plus G^T scaled by 2 with an extra all-ones/2
    bias row so PSUM holds 2*d + 2. Then a single DVE op per tile:
        out = (psum mod 4) - 1  ->  {+1 even d, -1 odd d}
  - Output written as bf16 (+-1 exact), cast to f32 on host.
"""

import numpy as np
import ml_dtypes

BF16 = ml_dtypes.bfloat16

B_FULL = 16384
K_MSG = 1200
N_BITS = 2400
N_CORES = 8
B_LOC = B_FULL // N_CORES  # 2048
K_PAD = 1280  # 10 k-tiles of 128; row 1200 is the +2 bias row
P = 128

_CACHE: dict = {}
# fp8 DoubleRow matmul (2 contraction rows per PE cell): compiles and is
# exact in CoreSim, but the generated NEFF hit NRT_EXEC_UNIT_UNRECOVERABLE
# on hardware — keep the proven bf16 path.
USE_DR = False


def _mm_np_dtype():
    if not USE_DR:
        return BF16
    import concourse.mybir as mybir
    return mybir.dt.np(mybir.dt.float8e4)


def _build(bl, k_msg, k_pad, n_par, n_bits, base_col, with_identity,
           use_dr=False):
    """Build + compile the per-core Bass program.

    bl: local batch rows; n_par: matmul output columns; base_col: where the
    matmul columns land in the output; with_identity: also emit
    out[:, :k_msg] = 1-2*m from a natural-layout copy of m.
    """
    import concourse.bacc as bacc
    import concourse.mybir as mybir
    import concourse.tile as tile

    bf16 = mybir.dt.bfloat16
    f32 = mybir.dt.float32
    i32 = mybir.dt.int32
    Alu = mybir.AluOpType
    Act = mybir.ActivationFunctionType

    nc = bacc.Bacc("TRN2", target_bir_lowering=False, debug=False,
                   num_devices=N_CORES)

    fp8 = mybir.dt.float8e4
    mm_dt = fp8 if use_dr else bf16
    mT = nc.dram_tensor("mT", [k_pad, bl], mm_dt, kind="ExternalInput")
    gT = nc.dram_tensor("GT2", [k_pad, n_par], mm_dt, kind="ExternalInput")
    out = nc.dram_tensor("out", [bl, n_bits], bf16, kind="ExternalOutput")
    mnat = None
    if with_identity:
        mnat = nc.dram_tensor("mnat", [bl, k_msg], bf16, kind="ExternalInput")

    k_step = 2 * P if use_dr else P
    kt_n = k_pad // k_step
    nb = bl // P
    chunks = []
    n0 = 0
    while n0 < n_par:
        w = min(512, n_par - n0)
        chunks.append((n0, w))
        n0 += w

    with tile.TileContext(nc) as tc:
        with (
            tc.tile_pool(name="const", bufs=1) as cpool,
            tc.tile_pool(name="mn", bufs=3) as mnpool,
            tc.tile_pool(name="po", bufs=6) as popool,
            tc.tile_pool(name="io", bufs=3) as iopool,
            tc.tile_pool(name="ps", bufs=6, space="PSUM") as pspool,
        ):
            gts, mts = [], []
            for t in range(kt_n):
                ks = slice(t * k_step, (t + 1) * k_step)
                if use_dr:
                    # [2*P, X] DRAM rows -> [P, 2, X] SBUF (k = t*256 + i*128 + p)
                    gt_t = cpool.tile([P, 2, n_par], mm_dt, tag=f"gt{t}")
                    nc.sync.dma_start(
                        out=gt_t[:],
                        in_=gT[ks, :].rearrange("(i p) c -> p i c", i=2))
                    mt_t = cpool.tile([P, 2, bl], mm_dt, tag=f"mt{t}")
                    nc.sync.dma_start(
                        out=mt_t[:],
                        in_=mT[ks, :].rearrange("(i p) c -> p i c", i=2))
                else:
                    gt_t = cpool.tile([P, n_par], mm_dt, tag=f"gt{t}")
                    nc.sync.dma_start(out=gt_t[:], in_=gT[ks, :])
                    mt_t = cpool.tile([P, bl], mm_dt, tag=f"mt{t}")
                    nc.sync.dma_start(out=mt_t[:], in_=mT[ks, :])
                gts.append(gt_t)
                mts.append(mt_t)

            for b in range(nb):
                bs = slice(b * P, (b + 1) * P)
                psts = [pspool.tile([P, 512], f32, tag="ps", name=f"ps{b}_{ci}")
                        for ci in range(len(chunks))]
                for t in range(kt_n):
                    for ci, (n0, w) in enumerate(chunks):
                        if use_dr:
                            nc.tensor.matmul(
                                psts[ci][:, :w],
                                mts[t][:, :, bs],
                                gts[t][:, :, n0:n0 + w],
                                start=(t == 0),
                                stop=(t == kt_n - 1),
                                perf_mode=mybir.MatmulPerfMode.DoubleRow,
                            )
                        else:
                            nc.tensor.matmul(
                                psts[ci][:, :w],
                                mts[t][:, bs],
                                gts[t][:, n0:n0 + w],
                                start=(t == 0),
                                stop=(t == kt_n - 1),
                            )
                for ci, (n0, w) in enumerate(chunks):
                    # parity -> BPSK: p = int(d) & 1 ; out = -2p + 1
                    it = popool.tile([P, 512], i32, tag="pi",
                                     name=f"pi{b}_{ci}")
                    nc.vector.tensor_copy(it[:, :w], psts[ci][:, :w])
                    pt = popool.tile([P, 512], i32, tag="pp",
                                     name=f"pp{b}_{ci}")
                    nc.vector.tensor_scalar(
                        pt[:, :w], it[:, :w], 1, None, op0=Alu.bitwise_and,
                    )
                    ot = popool.tile([P, 512], bf16, tag="po",
                                     name=f"po{b}_{ci}")
                    nc.vector.tensor_scalar(
                        ot[:, :w], pt[:, :w], -2.0, 1.0,
                        op0=Alu.mult, op1=Alu.add,
                    )
                    nc.sync.dma_start(
                        out=out[bs, base_col + n0:base_col + n0 + w],
                        in_=ot[:, :w],
                    )
                if with_identity:
                    mn = mnpool.tile([P, k_msg], bf16, tag="mn")
                    nc.sync.dma_start(out=mn[:], in_=mnat[bs, :])
                    io = iopool.tile([P, k_msg], bf16, tag="io")
                    nc.vector.tensor_scalar(
                        io[:], mn[:], -2.0, 1.0, op0=Alu.mult, op1=Alu.add,
                    )
                    nc.sync.dma_start(out=out[bs, 0:k_msg], in_=io[:])

    nc.compile()
    return nc


def _get_nc(fast: bool):
    key = ("fast" if fast else "full", USE_DR)
    if key not in _CACHE:
        if fast:
            _CACHE[key] = _build(B_LOC, K_MSG, K_PAD, N_BITS - K_MSG, N_BITS,
                                 K_MSG, True, use_dr=USE_DR)
        else:
            _CACHE[key] = _build(B_LOC, K_MSG, K_PAD, N_BITS, N_BITS, 0, False,
                                 use_dr=USE_DR)
    return _CACHE[key]


def _prep_inputs(m, G, fast: bool):
    """Host-side marshaling: casts, transposes, padding, bias row."""
    mm_dt = _mm_np_dtype()
    m_mm = m.astype(mm_dt)
    if fast:
        g_rows = G[K_MSG:N_BITS]  # parity rows only
    else:
        g_rows = G
    n_par = g_rows.shape[0]
    gT2 = np.zeros((K_PAD, n_par), dtype=mm_dt)
    gT2[:K_MSG] = g_rows.T.astype(mm_dt)  # psum = d (count of set bits)

    in_maps = []
    for c in range(N_CORES):
        m_c = m_mm[c * B_LOC:(c + 1) * B_LOC]
        mT = np.zeros((K_PAD, B_LOC), dtype=mm_dt)
        mT[:K_MSG] = np.ascontiguousarray(m_c.T)
        im = {"mT": mT, "GT2": gT2}
        if fast:
            im["mnat"] = np.ascontiguousarray(
                m[c * B_LOC:(c + 1) * B_LOC].astype(BF16))
        in_maps.append(im)
    return in_maps


def _run(m, G, trace=False):
    from concourse.bass_utils import run_bass_kernel_spmd

    fast = bool(
        np.array_equal(G[:K_MSG], np.eye(K_MSG, dtype=G.dtype))
        and ((G == 0) | (G == 1)).all()
    )
    nc = _get_nc(fast)
    in_maps = _prep_inputs(m, G, fast)
    res = run_bass_kernel_spmd(
        nc, in_maps, core_ids=list(range(N_CORES)), trace=trace,
    )
    parts = [res.results[c]["out"] for c in range(N_CORES)]
    full = np.concatenate(parts, axis=0).astype(np.float32)
    return full, res


def kernel(m, G, snr=None):
    m = np.asarray(m)
    G = np.asarray(G)
    full, _ = _run(m, G, trace=False)
    return full



# revision 23
# speedup vs baseline: 80.9821x; 80.9821x over previous
"""LDPC encoder kernel for Trainium2 (8 NeuronCores, batch-sharded).

Computes out = 1 - 2*((m @ G^T) mod 2)  (BPSK-mapped LDPC codeword).

  m: [16384, 1200] int32 (0/1)   G: [2400, 1200] float32 (0/1)
  out: [16384, 2400] float32 (+-1)

Strategy:
  - Shard the batch over 8 cores (2048 rows each); G replicated.
  - G is systematic (G[:1200] == I), so out[:, :1200] = 1 - 2*m is a pure
    elementwise map (done on the otherwise-idle GPSIMD engine); only
    the 1200 parity columns need a matmul.
  - Matmul in fp8e4 (values 0/1 are exact; PSUM accumulates fp32
    exactly, psum = d = bit count). Host feeds m transposed ([K,B]
    layout) so the stationary operand needs no on-device transpose.
    Parity+BPSK post-processing (cast psum f32->i16, p = &1,
    out = 1 - 2*p) is spread across DVE/ACT/GPSIMD so no single engine
    exceeds the PE's per-tile cadence. (DVE/GPSIMD have no mod op, and
    ACT Sin has no range reduction -- both verified on HW.)
  - Output written as fp8e4 (+-1 exact), cast to f32 on host.
  - USE_DR=True packs contraction rows in pairs (MatmulPerfMode.DoubleRow)
    for ~1.4x tensor-engine throughput.
"""

import numpy as np
import ml_dtypes

BF16 = ml_dtypes.bfloat16

B_FULL = 16384
K_MSG = 1200
N_BITS = 2400
N_CORES = 8
B_LOC = B_FULL // N_CORES  # 2048
K_PAD = 1280  # zero-padded to 5 DoubleRow k-pair-tiles of 256
P = 128

_CACHE: dict = {}
USE_DR = True


def _mm_np_dtype():
    import concourse.mybir as mybir
    return mybir.dt.np(mybir.dt.float8e4)


def _build(bl, k_msg, k_pad, n_par, n_bits, base_col, with_identity,
           use_dr=False):
    """Build + compile the per-core Bass program.

    bl: local batch rows; n_par: matmul output columns; base_col: where the
    matmul columns land in the output; with_identity: also emit
    out[:, :k_msg] = 1-2*m from a natural-layout copy of m.
    """
    import concourse.bacc as bacc
    import concourse.mybir as mybir
    import concourse.tile as tile

    f32 = mybir.dt.float32
    i16 = mybir.dt.int16
    fp8 = mybir.dt.float8e4
    Alu = mybir.AluOpType
    Act = mybir.ActivationFunctionType

    nc = bacc.Bacc("TRN2", target_bir_lowering=False, debug=False,
                   num_devices=N_CORES)

    k_step = 2 * P if use_dr else P
    kt_n = k_pad // k_step
    if use_dr:
        # paired layout: DRAM row (t*P + p) = concat(x[2P*t + p], x[2P*t + P + p])
        mT = nc.dram_tensor("mT", [kt_n * P, 2 * bl], fp8, kind="ExternalInput")
        gT = nc.dram_tensor("GT2", [kt_n * P, 2 * n_par], fp8,
                            kind="ExternalInput")
    else:
        mT = nc.dram_tensor("mT", [k_pad, bl], fp8, kind="ExternalInput")
        gT = nc.dram_tensor("GT2", [k_pad, n_par], fp8, kind="ExternalInput")
    out = nc.dram_tensor("out", [bl, n_bits], fp8, kind="ExternalOutput")
    mnat = None
    if with_identity:
        mnat = nc.dram_tensor("mnat", [bl, k_msg], fp8, kind="ExternalInput")

    nb = bl // P
    chunks = []
    n0 = 0
    while n0 < n_par:
        w = min(512, n_par - n0)
        chunks.append((n0, w))
        n0 += w

    with tile.TileContext(nc) as tc:
        with (
            tc.tile_pool(name="const", bufs=1) as cpool,
            tc.tile_pool(name="mn", bufs=4) as mnpool,
            tc.tile_pool(name="po", bufs=6) as popool,
            tc.tile_pool(name="io", bufs=4) as iopool,
            tc.tile_pool(name="ps", bufs=8, space="PSUM") as pspool,
        ):
            # Warm up the PE clock during the DMA prologue: the HAM clock
            # gate starts at 1.2GHz and needs ~3.4us of sustained activity
            # to release to 2.4GHz. ~48 dummy matmuls on a memset tile keep
            # the PE busy while the const loads stream, so the real matmuls
            # start (and stay) at full clock instead of ramping mid-kernel.
            wt = cpool.tile([P, 64], fp8, tag="warm")
            nc.vector.memset(wt[:], 1.0)
            wps = pspool.tile([P, 512], f32, tag="ps", name="warmps")
            for _ in range(48):
                nc.tensor.matmul(wps[0:64, 0:64], wt[:], wt[:],
                                 start=True, stop=True)

            # const loads split across both HWDGE rings: mt (big) on sync,
            # gt on scalar -- the rings transfer concurrently (~360GB/s each)
            gts, mts = [], []
            for t in range(kt_n):
                rs = slice(t * P, (t + 1) * P)
                if use_dr:
                    gt_t = cpool.tile([P, 2, n_par], fp8, tag=f"gt{t}")
                    nc.scalar.dma_start(out=gt_t[:], in_=gT[rs, :])
                    mt_t = cpool.tile([P, 2, bl], fp8, tag=f"mt{t}")
                    nc.sync.dma_start(out=mt_t[:], in_=mT[rs, :])
                else:
                    gt_t = cpool.tile([P, n_par], fp8, tag=f"gt{t}")
                    nc.scalar.dma_start(out=gt_t[:], in_=gT[rs, :])
                    mt_t = cpool.tile([P, bl], fp8, tag=f"mt{t}")
                    nc.sync.dma_start(out=mt_t[:], in_=mT[rs, :])
                gts.append(gt_t)
                mts.append(mt_t)

            for b in range(nb):
                bs = slice(b * P, (b + 1) * P)
                psts = [pspool.tile([P, 512], f32, tag="ps", name=f"ps{b}_{ci}")
                        for ci in range(len(chunks))]
                for t in range(kt_n):
                    for ci, (n0, w) in enumerate(chunks):
                        if use_dr:
                            nc.tensor.matmul(
                                psts[ci][:, :w],
                                mts[t][:, :, bs],
                                gts[t][:, :, n0:n0 + w],
                                start=(t == 0),
                                stop=(t == kt_n - 1),
                                perf_mode=mybir.MatmulPerfMode.DoubleRow,
                            )
                        else:
                            nc.tensor.matmul(
                                psts[ci][:, :w],
                                mts[t][:, bs],
                                gts[t][:, n0:n0 + w],
                                start=(t == 0),
                                stop=(t == kt_n - 1),
                            )
                # all 2400 output cols assembled in one fp8 buffer, one DMA.
                # post chain per chunk: cast f32->i16, p = &1, out = 1-2p.
                # Work is spread so every engine stays under the PE's
                # ~2.7us/b-tile cadence (DVE alone would be co-critical).
                ob = iopool.tile([P, n_bits], fp8, tag="ob", name=f"ob{b}")
                for ci, (n0, w) in enumerate(chunks):
                    it = popool.tile([P, 512], i16, tag="pi",
                                     name=f"pi{b}_{ci}")
                    if ci % 3 == 2:
                        nc.scalar.activation(
                            it[:, :w], psts[ci][:, :w], Act.Copy,
                        )
                    else:
                        nc.vector.tensor_copy(it[:, :w], psts[ci][:, :w])
                    pt = popool.tile([P, 512], i16, tag="pp",
                                     name=f"pp{b}_{ci}")
                    nc.vector.tensor_scalar(
                        pt[:, :w], it[:, :w], 1, None, op0=Alu.bitwise_and,
                    )
                    osl = ob[:, base_col + n0:base_col + n0 + w]
                    if ci % 3 == 1:
                        nc.gpsimd.tensor_scalar(
                            osl, pt[:, :w], -2.0, 1.0,
                            op0=Alu.mult, op1=Alu.add,
                        )
                    else:
                        nc.scalar.activation(
                            osl, pt[:, :w], Act.Identity, bias=1.0, scale=-2.0,
                        )
                if with_identity:
                    mn = mnpool.tile([P, k_msg], fp8, tag="mn")
                    nc.sync.dma_start(out=mn[:], in_=mnat[bs, :])
                    nc.gpsimd.tensor_scalar(
                        ob[:, 0:k_msg], mn[:], -2.0, 1.0,
                        op0=Alu.mult, op1=Alu.add,
                    )
                nc.sync.dma_start(out=out[bs, :], in_=ob[:])

    nc.compile()
    return nc


def _get_nc(fast: bool):
    key = ("fast" if fast else "full", USE_DR)
    if key not in _CACHE:
        if fast:
            _CACHE[key] = _build(B_LOC, K_MSG, K_PAD, N_BITS - K_MSG, N_BITS,
                                 K_MSG, True, use_dr=USE_DR)
        else:
            _CACHE[key] = _build(B_LOC, K_MSG, K_PAD, N_BITS, N_BITS, 0, False,
                                 use_dr=USE_DR)
    return _CACHE[key]


def _pair_rows(a):
    """[K_PAD, X] -> [K_PAD//2, 2*X]: row t*128+p = concat(a[256t+p], a[256t+128+p])."""
    kp, x = a.shape
    return np.ascontiguousarray(
        a.reshape(kp // 256, 2, P, x).transpose(0, 2, 1, 3).reshape(kp // 2, 2 * x)
    )


def _prep_inputs(m, G, fast: bool):
    """Host-side marshaling: fp8 casts, transposes, padding, DR pairing."""
    mm_dt = _mm_np_dtype()
    if fast:
        g_rows = G[K_MSG:N_BITS]  # parity rows only
    else:
        g_rows = G
    n_par = g_rows.shape[0]
    gT2 = np.zeros((K_PAD, n_par), dtype=mm_dt)
    gT2[:K_MSG] = g_rows.T.astype(mm_dt)  # psum = d (count of set bits)
    if USE_DR:
        gT2 = _pair_rows(gT2)

    m_mm = m.astype(mm_dt)
    in_maps = []
    for c in range(N_CORES):
        m_c = m_mm[c * B_LOC:(c + 1) * B_LOC]
        mT = np.zeros((K_PAD, B_LOC), dtype=mm_dt)
        mT[:K_MSG] = np.ascontiguousarray(m_c.T)
        if USE_DR:
            mT = _pair_rows(mT)
        im = {"mT": mT, "GT2": gT2}
        if fast:
            im["mnat"] = np.ascontiguousarray(m_c)
        in_maps.append(im)
    return in_maps


def _run(m, G, trace=False):
    from concourse.bass_utils import run_bass_kernel_spmd

    binary = bool(((G == 0) | (G == 1)).all())
    if not binary:
        # exact host fallback for arbitrary G (never hit by the LDPC
        # encoder's binary systematic G)
        d = np.mod(m.astype(np.float64) @ G.T.astype(np.float64), 2.0)
        return (1.0 - 2.0 * d).astype(np.float32), None
    fast = bool(np.array_equal(G[:K_MSG], np.eye(K_MSG, dtype=G.dtype)))
    nc = _get_nc(fast)
    in_maps = _prep_inputs(m, G, fast)
    res = run_bass_kernel_spmd(
        nc, in_maps, core_ids=list(range(N_CORES)), trace=trace,
    )
    parts = [res.results[c]["out"] for c in range(N_CORES)]
    full = np.concatenate(parts, axis=0).astype(np.float32)
    return full, res


def kernel(m, G, snr=None):
    m = np.asarray(m)
    G = np.asarray(G)
    full, _ = _run(m, G, trace=False)
    return full
